# revision 65
# baseline (speedup 1.0000x reference)
"""Trainium2 Bass kernel for CustomGPT2Attention (B=4, S=2048, D=1024, H=16).

Strategy: tensor-parallel over heads. Each of the 8 NeuronCores owns 2 heads
(a 128-wide slice of the QKV projections and the matching 128 rows of Wo),
computes its partial output projection over the full batch, and the host sums
the 8 partials (the "all-reduce" of the row-parallel c_proj) plus bo.

All on-device layouts are chosen so no transposes are ever needed:
  - host ships hidden^T [D, B*S] (bf16)
  - QT, KT come out of the projection as [hd_local, B*S]
  - scores are computed transposed, ST[kv, q] = K^T Q, softmax runs along
    the partition (kv) axis using matmul-with-ones for the denominators
  - AV output OT[hd_local, q] is directly the lhsT of the output projection
Compute dtype bf16 (fp32 PSUM accumulation everywhere).
"""

import collections
import os
import sys

for _p in ("/opt/trn_rl_repo", "/root/.axon_site/_ro/trn_rl_repo"):
    if os.path.isdir(_p) and _p not in sys.path:
        sys.path.insert(0, _p)

import numpy as np
import ml_dtypes

BF16 = ml_dtypes.bfloat16

B, S, D, H, HD = 4, 2048, 1024, 16, 64
BS = B * S            # 8192 tokens
NCORES = 8
DL = D // NCORES      # 128 = per-core slice (2 heads x 64)
NK = D // 128         # 8 contraction chunks for the projections
SQ = 512              # q free-block width
NJ = S // SQ          # 4 q-blocks per batch
NT = BS // 128        # 64 s-tiles (of 128 tokens)
SCALE = 1.0 / 8.0     # 1/sqrt(HD)

_CACHE = {}
LAST_RESULTS = None
KDEBUG = bool(os.environ.get("KDEBUG"))


def _build_nc():
    import concourse.bacc as bacc
    import concourse.tile as tile
    import concourse.mybir as mybir
    import bass_rust

    dt = mybir.dt
    AF = mybir.ActivationFunctionType

    class _Bacc(bacc.Bacc):
        # All ACT functions we use (Exp, Ln, Identity, Copy) live in the
        # natural_log_exp_and_others table set. The stock table-load pass
        # assigns each function its first matching set, which thrashes
        # ACT_TABLE_LOADs (~1.3us each) between exp/ln sets inside the
        # softmax loop. Restrict the pass to the one set that has them all.
        def insert_act_table_loads(self):
            from concourse.hw_specs import get_activation_tables
            has_activation = any(
                isinstance(i, mybir.InstActivation)
                for b in self.main_func.blocks
                for i in b.instructions
            )
            if not has_activation:
                return
            tables = []
            for name, funcs in get_activation_tables(self.m.arch).items():
                if name != "natural_log_exp_and_others":
                    funcs = set()
                tables.append((name, funcs))
            bass_rust.insert_act_table_loads(self, tables)

    nc = _Bacc(
        "TRN2", target_bir_lowering=False, debug=False, num_devices=NCORES
    )

    ht_d = nc.dram_tensor("ht", [D, BS], dt.bfloat16, kind="ExternalInput").ap()
    wq_d = nc.dram_tensor("wq", [D, DL], dt.bfloat16, kind="ExternalInput").ap()
    wk_d = nc.dram_tensor("wk", [D, DL], dt.bfloat16, kind="ExternalInput").ap()
    wv_d = nc.dram_tensor("wv", [D, DL], dt.bfloat16, kind="ExternalInput").ap()
    wo_d = nc.dram_tensor("wo", [DL, D], dt.bfloat16, kind="ExternalInput").ap()
    bq_d = nc.dram_tensor("bq", [DL, 1], dt.float32, kind="ExternalInput").ap()
    bk_d = nc.dram_tensor("bk", [DL, 1], dt.float32, kind="ExternalInput").ap()
    bvb_d = nc.dram_tensor("bvb", [DL, 1], dt.float32, kind="ExternalInput").ap()
    id_d = nc.dram_tensor("ident", [128, 128], dt.bfloat16, kind="ExternalInput").ap()
    mk_d = nc.dram_tensor("mask", [128, 4 * 1024], dt.bfloat16, kind="ExternalInput").ap()
    out_d = nc.dram_tensor("out", [BS, D], dt.float32, kind="ExternalOutput").ap()

    taps = None
    if KDEBUG:
        taps = {
            "dqt": nc.dram_tensor("dqt", [128, BS], dt.bfloat16, kind="ExternalOutput").ap(),
            "dkt": nc.dram_tensor("dkt", [128, BS], dt.bfloat16, kind="ExternalOutput").ap(),
            "dv": nc.dram_tensor("dv", [128, BS], dt.bfloat16, kind="ExternalOutput").ap(),
            "dot": nc.dram_tensor("dot", [128, BS], dt.bfloat16, kind="ExternalOutput").ap(),
            "dav": nc.dram_tensor("dav", [128, 512], dt.float32, kind="ExternalOutput").ap(),
            "dden": nc.dram_tensor("dden", [128, 512], dt.float32, kind="ExternalOutput").ap(),
            "drc": nc.dram_tensor("drc", [128, 512], dt.float32, kind="ExternalOutput").ap(),
            "dbc": nc.dram_tensor("dbc", [128, 512], dt.float32, kind="ExternalOutput").ap(),
            "dpt": nc.dram_tensor("dpt", [128, 1024], dt.bfloat16, kind="ExternalOutput").ap(),
        }

    with tile.TileContext(nc) as tc:
        _body(tc, nc, mybir, ht_d, wq_d, wk_d, wv_d, wo_d, bq_d, bk_d, bvb_d,
              id_d, mk_d, out_d, taps)

    nc.compile()
    return nc


def _body(tc, nc, mybir, ht_d, wq_d, wk_d, wv_d, wo_d, bq_d, bk_d, bvb_d,
          id_d, mk_d, out_d, taps=None):
    from contextlib import ExitStack

    dt = mybir.dt
    AF = mybir.ActivationFunctionType

    ctx = ExitStack()
    with ctx:
        consts = ctx.enter_context(tc.tile_pool(name="consts", bufs=1))

        # --- constants / weights (persist whole kernel) ---
        wq_sb = consts.tile([128, D], dt.bfloat16, name="wq_sb")
        wk_sb = consts.tile([128, D], dt.bfloat16, name="wk_sb")
        wv_sb = consts.tile([128, D], dt.bfloat16, name="wv_sb")
        wo_sb = consts.tile([128, D], dt.bfloat16, name="wo_sb")
        bq_sb = consts.tile([128, 1], dt.float32, name="bq_sb")
        bk_sb = consts.tile([128, 1], dt.float32, name="bk_sb")
        bvb_sb = consts.tile([128, 1], dt.float32, name="bvb_sb")
        id_sb = consts.tile([128, 128], dt.bfloat16, name="id_sb")
        mask_sb = consts.tile([128, 4 * 1024], dt.bfloat16, name="mask_sb")
        ones_bf = consts.tile([128, 1], dt.bfloat16, name="ones_bf")
        ones_f32 = consts.tile([128, 64], dt.float32, name="ones_f32")

        # weights are [D, DL] in DRAM; load as 8 lhsT tiles [128, 128] side by
        # side -> SBUF [128, 8*128]
        for w_d, w_sb in ((wq_d, wq_sb), (wk_d, wk_sb), (wv_d, wv_sb)):
            nc.sync.dma_start(
                w_sb.rearrange("p (k n) -> p k n", k=NK),
                w_d.rearrange("(k p) n -> p k n", p=128),
            )
        nc.sync.dma_start(wo_sb[:, :], wo_d[:, :])
        nc.sync.dma_start(bq_sb[:, :], bq_d[:, :])
        nc.sync.dma_start(bk_sb[:, :], bk_d[:, :])
        nc.sync.dma_start(bvb_sb[:, :], bvb_d[:, :])
        nc.sync.dma_start(id_sb[:, :], id_d[:, :])
        nc.sync.dma_start(mask_sb[:, :], mk_d[:, :])
        nc.gpsimd.memset(ones_bf[:, :], 1.0)
        nc.gpsimd.memset(ones_f32[:, :], 1.0)

        # --- persistent activation tensors ---
        qt_sb = consts.tile([128, BS], dt.bfloat16, name="qt_sb")   # Q^T
        kt_sb = consts.tile([128, BS], dt.bfloat16, name="kt_sb")   # K^T
        vt_sb = consts.tile([128, BS], dt.bfloat16, name="vt_sb")   # V^T
        v_sb = consts.tile([128, BS], dt.bfloat16, name="v_sb")     # V, s-tiles side by side: tile t at cols [t*128, t*128+128)
        ot_sb = consts.tile([128, BS], dt.bfloat16, name="ot_sb")   # attn out ^T

        # single PSUM pool so all phases can interleave:
        #   proj: 2x [128,512] banks (QKV projection accumulators)
        #   st:   2x [128,1024] (scores / bc / out-proj psum)
        #   avden:2x [128,512] (attn-out accumulator + denominators)
        ps = ctx.enter_context(tc.tile_pool(name="ps", bufs=1, space="PSUM"))
        hpool = ctx.enter_context(tc.tile_pool(name="hpool", bufs=10))
        ptpool = ctx.enter_context(tc.tile_pool(name="ptpool", bufs=3))
        rcpool = ctx.enter_context(tc.tile_pool(name="rcpool", bufs=2))
        bcpool = ctx.enter_context(tc.tile_pool(name="bcpool", bufs=2))
        obpool = ctx.enter_context(tc.tile_pool(name="obpool", bufs=3))

        ht_tiles = {}

        def dma_a(bi, split=False):
            hts = []
            for k in range(NK):
                ht_t = hpool.tile([128, S], dt.bfloat16, name=f"ht_{bi}_{k}",
                                  tag="ht", bufs=14)
                if split:
                    # batch 0 gates kernel startup: land the first q-block's
                    # columns early so the first projection chain can start
                    # before the whole batch has arrived
                    for h in range(2):
                        nc.sync.dma_start(
                            ht_t[:, h * (S // 2):(h + 1) * (S // 2)],
                            ht_d[k * 128:(k + 1) * 128,
                                 bi * S + h * (S // 2):bi * S + (h + 1) * (S // 2)])
                else:
                    nc.sync.dma_start(ht_t[:, :], ht_d[k * 128:(k + 1) * 128,
                                                       bi * S:(bi + 1) * S])
                hts.append(ht_t)
            ht_tiles[bi] = hts

        def phase_a_group(bi, sub):
            """QKV projection + V transposes for one 512-token sub-block;
            everything phase_b_j(bi, j=sub) needs beyond earlier subs."""
            hts = ht_tiles[bi]
            cols = slice(bi * S + sub * SQ, bi * S + (sub + 1) * SQ)
            for w_sb, b_sb, o_sb in ((wq_sb, bq_sb, qt_sb),
                                     (wk_sb, bk_sb, kt_sb),
                                     (wv_sb, bvb_sb, vt_sb)):
                pj_ps = ps.tile([128, SQ], dt.float32, tag="proj", bufs=2,
                                name="pj_ps")
                for k in range(NK):
                    nc.tensor.matmul(
                        pj_ps[:, :], w_sb[:, k * 128:(k + 1) * 128],
                        hts[k][:, sub * SQ:(sub + 1) * SQ],
                        start=(k == 0), stop=(k == NK - 1))
                nc.vector.tensor_scalar_add(o_sb[:, cols], pj_ps[:, :],
                                            b_sb[:, 0:1])
            # transpose this sub's V^T back to V [s, hd] via the PE
            for st in range(sub * 4, sub * 4 + 4):
                g = bi * (S // 128) + st
                vtp = ps.tile([128, 128], dt.bfloat16, tag="proj", bufs=2)
                nc.tensor.transpose(
                    vtp[:, :], vt_sb[:, g * 128:(g + 1) * 128], id_sb[:, :])
                nc.vector.tensor_copy(v_sb[:, g * 128:(g + 1) * 128],
                                      vtp[:, :])
            if sub == NJ - 1:
                ht_tiles.pop(bi)

        def phase_b_j(bi, j):
            # attention for the 2 local heads of batch bi, q-block j
            if True:
                qcols = slice(bi * S + j * SQ, bi * S + (j + 1) * SQ)
                nk = 4 * j + 4
                av_ps = ps.tile([128, SQ], dt.float32, tag="avden", bufs=2)
                den_ps = ps.tile([128, SQ], dt.float32, tag="avden", bufs=2)
                # software-pipeline with a lag so the AV matmuls' exp
                # dependencies are already satisfied when the PE reaches
                # them (keeps the PE stream back-to-back)
                LAG = 4
                pts = {}
                for kk in range(nk + LAG):
                    if kk < nk:
                        k = kk
                        kvc = slice(bi * S + k * 128, bi * S + (k + 1) * 128)
                        st_ps = ps.tile([128, 2 * SQ], dt.float32, tag="st",
                                        bufs=2)
                        # scores^T for both heads, row-packed (K=64 each)
                        nc.tensor.matmul(st_ps[:, 0:SQ], kt_sb[0:64, kvc],
                                         qt_sb[0:64, qcols],
                                         start=True, stop=True)
                        nc.tensor.matmul(st_ps[:, SQ:2 * SQ],
                                         kt_sb[64:128, kvc],
                                         qt_sb[64:128, qcols],
                                         start=True, stop=True)
                        pt = ptpool.tile([128, 2 * SQ], dt.bfloat16,
                                         tag="pt", bufs=6)
                        pts[k] = pt
                        delta = (k - (nk - 4)) * 128 if k >= nk - 4 else 0
                        if delta >= 256:
                            # columns < delta are fully masked: skip their
                            # exp (mask multiply below zeroes them; pt slots
                            # only ever hold finite stale values)
                            nc.scalar.activation(pt[:, delta:SQ],
                                                 st_ps[:, delta:SQ], AF.Exp,
                                                 scale=SCALE)
                            nc.scalar.activation(pt[:, SQ + delta:2 * SQ],
                                                 st_ps[:, SQ + delta:2 * SQ],
                                                 AF.Exp, scale=SCALE)
                        else:
                            nc.scalar.activation(pt[:, :], st_ps[:, :],
                                                 AF.Exp, scale=SCALE)
                        if k >= nk - 4:
                            midx = k - (nk - 4)
                            nc.vector.tensor_mul(
                                pt[:, :], pt[:, :],
                                mask_sb[:, midx * 1024:(midx + 1) * 1024])
                        if taps is not None and bi == 0 and j == 0 and k == 0:
                            nc.sync.dma_start(taps["dpt"][:, :], pt[:, :])
                    if kk >= LAG:
                        k = kk - LAG
                        pt = pts.pop(k)
                        # AV col-packed: head A -> rows 0:64, head B -> rows
                        # 64:128 of one bank; denominators -> rows 0 / 32 of
                        # a second bank. Same-bank groups are partition-
                        # disjoint; sim's group check is partition-collapsed
                        # -> skip.
                        g = bi * (S // 128) + k
                        va = v_sb[:, g * 128:g * 128 + 64]
                        vb = v_sb[:, g * 128 + 64:g * 128 + 128]
                        first, last = (k == 0), (k == nk - 1)
                        nc.tensor.matmul(av_ps[0:64, 0:SQ], va, pt[:, 0:SQ],
                                         start=first, stop=last)
                        nc.tensor.matmul(av_ps[64:128, 0:SQ], vb,
                                         pt[:, SQ:2 * SQ],
                                         start=first, stop=last,
                                         skip_group_check=True)
                        nc.tensor.matmul(den_ps[0:1, 0:SQ], ones_bf[:, 0:1],
                                         pt[:, 0:SQ], start=first, stop=last)
                        nc.tensor.matmul(den_ps[32:33, 0:SQ],
                                         ones_bf[:, 0:1], pt[:, SQ:2 * SQ],
                                         start=first, stop=last,
                                         skip_group_check=True)

                if taps is not None and bi == 0 and j == 0:
                    davs = rcpool.tile([128, SQ], dt.float32, tag="dav", bufs=1)
                    nc.scalar.copy(davs[:, :], av_ps[:, :])
                    nc.sync.dma_start(taps["dav"][:, :], davs[:, :])
                    ddens = rcpool.tile([128, SQ], dt.float32, tag="dden", bufs=1)
                    nc.scalar.copy(ddens[:, :], den_ps[:, :])
                    nc.sync.dma_start(taps["dden"][:, :], ddens[:, :])

                # ---- softmax normalization: 1/den = exp(-ln(den)) on ACT
                # (same table set as the softmax exp) ----
                rc = rcpool.tile([128, SQ], dt.float32, tag="rc", bufs=2)
                sc = rcpool.tile([128, SQ], dt.float32, tag="sc", bufs=2)
                nc.scalar.activation(sc[0:1, 0:SQ], den_ps[0:1, 0:SQ], AF.Ln)
                nc.scalar.activation(rc[0:1, 0:SQ], sc[0:1, 0:SQ],
                                     AF.Exp, scale=-1.0)
                nc.scalar.activation(sc[32:33, 0:SQ], den_ps[32:33, 0:SQ],
                                     AF.Ln)
                nc.scalar.activation(rc[32:33, 0:SQ], sc[32:33, 0:SQ],
                                     AF.Exp, scale=-1.0)
                # broadcast 1/den across the head partitions: head A via the
                # (idle) GPSIMD engine -- its partition_broadcast only reads
                # physical partition 0, which is exactly where den_A's recip
                # lives; head B (partition 32) still needs the PE matmul.
                bc_sb = bcpool.tile([128, SQ], dt.float32, tag="bc", bufs=2)
                nc.gpsimd.partition_broadcast(bc_sb[0:64, 0:SQ],
                                              rc[0:1, 0:SQ])
                # head B's broadcast rides in the unused rows 64:128 of the
                # den bank (no extra PSUM slot, no st-slot contention)
                nc.tensor.matmul(den_ps[64:128, 0:SQ], ones_f32[32:33, :],
                                 rc[32:33, 0:SQ], start=True, stop=True,
                                 skip_group_check=True)
                nc.vector.tensor_copy(bc_sb[64:128, 0:SQ],
                                      den_ps[64:128, 0:SQ])
                nc.vector.tensor_mul(ot_sb[0:64, qcols], av_ps[0:64, 0:SQ],
                                     bc_sb[0:64, 0:SQ])
                nc.vector.tensor_mul(ot_sb[64:128, qcols],
                                     av_ps[64:128, 0:SQ],
                                     bc_sb[64:128, 0:SQ])
                if taps is not None and bi == 0 and j == 0:
                    nc.sync.dma_start(taps["drc"][:, :], rc[:, :])
                    nc.sync.dma_start(taps["dbc"][:, :], bc_sb[:, :])

        def phase_c_sp(bi, sp, on_act=False):
            # output projection for one pair of s-tiles of batch bi
            ob = obpool.tile([128, 2048], dt.float32, tag="ob", bufs=3)
            for half in range(2):
                t = bi * (S // 128) + sp * 2 + half
                op_ps = ps.tile([128, 1024], dt.float32, tag="st", bufs=2)
                lhs = ot_sb[:, t * 128:(t + 1) * 128]
                nc.tensor.matmul(op_ps[:, 0:512], lhs, wo_sb[:, 0:512],
                                 start=True, stop=True)
                nc.tensor.matmul(op_ps[:, 512:1024], lhs, wo_sb[:, 512:1024],
                                 start=True, stop=True)
                if on_act and half == 0:
                    # tail units: split the PSUM->SBUF copies across the
                    # by-then idle ScalarE to unblock the DVE
                    nc.scalar.copy(ob[:, 0:1024], op_ps[:, :])
                else:
                    nc.vector.tensor_copy(
                        ob[:, half * 1024:(half + 1) * 1024], op_ps[:, :])
            row0 = bi * S + sp * 256
            nc.sync.dma_start(
                out_d[row0:row0 + 256, :].rearrange("(a p) n -> p a n", p=128),
                ob.rearrange("p (a n) -> p a n", a=2),
            )

        # the first pt slots are read (dead columns) before being fully
        # written; clear them so stale SBUF garbage can't be inf/nan
        for _ in range(6):
            ptz = ptpool.tile([128, 2 * SQ], dt.bfloat16, tag="pt", bufs=6)
            nc.gpsimd.memset(ptz[:, :], 0)



        # Emission plan: the per-engine instruction streams are fixed at
        # compile time, so PE-dense filler must be emitted inside the
        # attention stream where the exp-dependency stalls happen. Each
        # q-block of attention gets exactly one projection sub-group,
        # emitted two q-blocks ahead of its consumer -- uniform filler
        # density across the whole kernel -- plus deferred out-proj units.
        groups = [(bi, sub) for bi in range(B) for sub in range(NJ)]
        dma_a(0, split=True)
        phase_a_group(0, 0)
        phase_a_group(0, 1)
        gidx = 2
        deferred_c = collections.deque()
        for bi in range(B):
            for j in range(NJ):
                if j == 0 and bi + 1 < B:
                    dma_a(bi + 1)
                gj = 4 * bi + j
                while gidx < len(groups) and \
                        groups[gidx][0] * 4 + groups[gidx][1] <= gj + 2:
                    phase_a_group(*groups[gidx])
                    gidx += 1
                phase_b_j(bi, j)
                deferred_c.append((bi, 2 * j))
                deferred_c.append((bi, 2 * j + 1))
                npop = (3, 3, 2, 0)[j] if gj >= 2 else 0
                for _ in range(min(npop, len(deferred_c))):
                    phase_c_sp(*deferred_c.popleft())
        while gidx < len(groups):
            phase_a_group(*groups[gidx])
            gidx += 1
        while deferred_c:
            phase_c_sp(*deferred_c.popleft(), on_act=True)

        if taps is not None:
            nc.sync.dma_start(taps["dqt"][:, :], qt_sb[:, :])
            nc.sync.dma_start(taps["dkt"][:, :], kt_sb[:, :])
            nc.sync.dma_start(taps["dv"][:, :], v_sb[:, :])
            nc.sync.dma_start(taps["dot"][:, :], ot_sb[:, :])


def _get_nc():
    if "nc" not in _CACHE:
        _CACHE["nc"] = _build_nc()
    return _CACHE["nc"]


def _build_mask():
    # mask[kv, q] for the 4 diagonal sub-tiles: delta = 0, 128, 256, 384.
    # allowed iff kv_local <= q_local - delta. Each [128, 512] block is
    # duplicated for the two heads -> [128, 1024] per delta, 4 deltas.
    i = np.arange(128)[:, None]
    q = np.arange(SQ)[None, :]
    blocks = []
    for delta in (0, 128, 256, 384):
        m = (i <= (q - delta)).astype(np.float32)
        blocks.append(np.concatenate([m, m], axis=1))
    return np.concatenate(blocks, axis=1).astype(BF16)


def kernel(hidden_states, Wq, bq, Wk, bk, Wv, bv, Wo, bo):
    global LAST_RESULTS
    from concourse import bass_utils

    nc = _get_nc()

    hid = np.ascontiguousarray(
        np.asarray(hidden_states, dtype=np.float32).reshape(BS, D).T)
    ht = hid.astype(BF16)
    mask = _build_mask()
    Wq = np.asarray(Wq, np.float32)
    Wk = np.asarray(Wk, np.float32)
    Wv = np.asarray(Wv, np.float32)
    Wo = np.asarray(Wo, np.float32)
    bq = np.asarray(bq, np.float32)
    bk = np.asarray(bk, np.float32)
    bv = np.asarray(bv, np.float32)
    bo = np.asarray(bo, np.float32)

    in_maps = []
    for c in range(NCORES):
        sl = slice(DL * c, DL * (c + 1))
        in_maps.append({
            "ht": ht,
            "wq": np.ascontiguousarray(Wq[:, sl]).astype(BF16),
            "wk": np.ascontiguousarray(Wk[:, sl]).astype(BF16),
            "wv": np.ascontiguousarray(Wv[:, sl]).astype(BF16),
            "wo": np.ascontiguousarray(Wo[sl, :]).astype(BF16),
            "bq": np.ascontiguousarray(bq[sl]).reshape(DL, 1),
            "bk": np.ascontiguousarray(bk[sl]).reshape(DL, 1),
            "bvb": np.ascontiguousarray(bv[sl]).reshape(DL, 1),
            "ident": np.eye(128, dtype=np.float32).astype(BF16),
            "mask": mask,
        })

    res = bass_utils.run_bass_kernel_spmd(
        nc, in_maps, core_ids=list(range(NCORES)))
    LAST_RESULTS = res

    out = res.results[0]["out"].astype(np.float32).copy()
    for c in range(1, NCORES):
        out += res.results[c]["out"]
    out += bo[None, :]
    return out.reshape(B, S, D)


# revision 66
# speedup vs baseline: 1.0486x; 1.0486x over previous
"""Trainium2 Bass kernel for CustomGPT2Attention (B=4, S=2048, D=1024, H=16).

Strategy: tensor-parallel over heads. Each of the 8 NeuronCores owns 2 heads
(a 128-wide slice of the QKV projections and the matching 128 rows of Wo),
computes its partial output projection over the full batch, and the host sums
the 8 partials (the "all-reduce" of the row-parallel c_proj) plus bo.

All on-device layouts are chosen so no transposes are ever needed:
  - host ships hidden^T [D, B*S] (bf16)
  - QT, KT come out of the projection as [hd_local, B*S]
  - scores are computed transposed, ST[kv, q] = K^T Q, softmax runs along
    the partition (kv) axis using matmul-with-ones for the denominators
  - AV output OT[hd_local, q] is directly the lhsT of the output projection
Compute dtype bf16 (fp32 PSUM accumulation everywhere).
"""

import collections
import os
import sys

for _p in ("/opt/trn_rl_repo", "/root/.axon_site/_ro/trn_rl_repo"):
    if os.path.isdir(_p) and _p not in sys.path:
        sys.path.insert(0, _p)

import numpy as np
import ml_dtypes

BF16 = ml_dtypes.bfloat16

B, S, D, H, HD = 4, 2048, 1024, 16, 64
BS = B * S            # 8192 tokens
NCORES = 8
DL = D // NCORES      # 128 = per-core slice (2 heads x 64)
NK = D // 128         # 8 contraction chunks for the projections
SQ = 512              # q free-block width
NJ = S // SQ          # 4 q-blocks per batch
NT = BS // 128        # 64 s-tiles (of 128 tokens)
SCALE = 1.0 / 8.0     # 1/sqrt(HD)

_CACHE = {}
LAST_RESULTS = None
KDEBUG = bool(os.environ.get("KDEBUG"))


def _build_nc():
    import concourse.bacc as bacc
    import concourse.tile as tile
    import concourse.mybir as mybir
    import bass_rust

    dt = mybir.dt
    AF = mybir.ActivationFunctionType

    class _Bacc(bacc.Bacc):
        # All ACT functions we use (Exp, Ln, Identity, Copy) live in the
        # natural_log_exp_and_others table set. The stock table-load pass
        # assigns each function its first matching set, which thrashes
        # ACT_TABLE_LOADs (~1.3us each) between exp/ln sets inside the
        # softmax loop. Restrict the pass to the one set that has them all.
        def insert_act_table_loads(self):
            from concourse.hw_specs import get_activation_tables
            has_activation = any(
                isinstance(i, mybir.InstActivation)
                for b in self.main_func.blocks
                for i in b.instructions
            )
            if not has_activation:
                return
            tables = []
            for name, funcs in get_activation_tables(self.m.arch).items():
                if name != "natural_log_exp_and_others":
                    funcs = set()
                tables.append((name, funcs))
            bass_rust.insert_act_table_loads(self, tables)

    nc = _Bacc(
        "TRN2", target_bir_lowering=False, debug=False, num_devices=NCORES
    )

    ht_d = nc.dram_tensor("ht", [D, BS], dt.bfloat16, kind="ExternalInput").ap()
    wq_d = nc.dram_tensor("wq", [D, DL], dt.bfloat16, kind="ExternalInput").ap()
    wk_d = nc.dram_tensor("wk", [D, DL], dt.bfloat16, kind="ExternalInput").ap()
    wv_d = nc.dram_tensor("wv", [D, DL], dt.bfloat16, kind="ExternalInput").ap()
    wo_d = nc.dram_tensor("wo", [DL, D], dt.bfloat16, kind="ExternalInput").ap()
    bq_d = nc.dram_tensor("bq", [DL, 1], dt.float32, kind="ExternalInput").ap()
    bk_d = nc.dram_tensor("bk", [DL, 1], dt.float32, kind="ExternalInput").ap()
    bvb_d = nc.dram_tensor("bvb", [DL, 1], dt.float32, kind="ExternalInput").ap()
    id_d = nc.dram_tensor("ident", [128, 128], dt.bfloat16, kind="ExternalInput").ap()
    mk_d = nc.dram_tensor("mask", [128, 4 * 1024], dt.bfloat16, kind="ExternalInput").ap()
    out_d = nc.dram_tensor("out", [BS, D], dt.float32, kind="ExternalOutput").ap()

    taps = None
    if KDEBUG:
        taps = {
            "dqt": nc.dram_tensor("dqt", [128, BS], dt.bfloat16, kind="ExternalOutput").ap(),
            "dkt": nc.dram_tensor("dkt", [128, BS], dt.bfloat16, kind="ExternalOutput").ap(),
            "dv": nc.dram_tensor("dv", [128, BS], dt.bfloat16, kind="ExternalOutput").ap(),
            "dot": nc.dram_tensor("dot", [128, BS], dt.bfloat16, kind="ExternalOutput").ap(),
            "dav": nc.dram_tensor("dav", [128, 512], dt.float32, kind="ExternalOutput").ap(),
            "dden": nc.dram_tensor("dden", [128, 512], dt.float32, kind="ExternalOutput").ap(),
            "drc": nc.dram_tensor("drc", [128, 512], dt.float32, kind="ExternalOutput").ap(),
            "dbc": nc.dram_tensor("dbc", [128, 512], dt.float32, kind="ExternalOutput").ap(),
            "dpt": nc.dram_tensor("dpt", [128, 1024], dt.bfloat16, kind="ExternalOutput").ap(),
        }

    with tile.TileContext(nc) as tc:
        _body(tc, nc, mybir, ht_d, wq_d, wk_d, wv_d, wo_d, bq_d, bk_d, bvb_d,
              id_d, mk_d, out_d, taps)

    nc.compile()
    return nc


def _body(tc, nc, mybir, ht_d, wq_d, wk_d, wv_d, wo_d, bq_d, bk_d, bvb_d,
          id_d, mk_d, out_d, taps=None):
    from contextlib import ExitStack

    dt = mybir.dt
    AF = mybir.ActivationFunctionType

    ctx = ExitStack()
    with ctx:
        consts = ctx.enter_context(tc.tile_pool(name="consts", bufs=1))

        # --- constants / weights (persist whole kernel) ---
        wq_sb = consts.tile([128, D], dt.bfloat16, name="wq_sb")
        wk_sb = consts.tile([128, D], dt.bfloat16, name="wk_sb")
        wv_sb = consts.tile([128, D], dt.bfloat16, name="wv_sb")
        wo_sb = consts.tile([128, D], dt.bfloat16, name="wo_sb")
        bq_sb = consts.tile([128, 1], dt.float32, name="bq_sb")
        bk_sb = consts.tile([128, 1], dt.float32, name="bk_sb")
        bvb_sb = consts.tile([128, 1], dt.float32, name="bvb_sb")
        id_sb = consts.tile([128, 128], dt.bfloat16, name="id_sb")
        mask_sb = consts.tile([128, 4 * 1024], dt.bfloat16, name="mask_sb")
        ones_bf = consts.tile([128, 1], dt.bfloat16, name="ones_bf")
        ones_f32 = consts.tile([128, 64], dt.float32, name="ones_f32")

        # weights are [D, DL] in DRAM; load as 8 lhsT tiles [128, 128] side by
        # side -> SBUF [128, 8*128]
        for w_d, w_sb in ((wq_d, wq_sb), (wk_d, wk_sb), (wv_d, wv_sb)):
            nc.sync.dma_start(
                w_sb.rearrange("p (k n) -> p k n", k=NK),
                w_d.rearrange("(k p) n -> p k n", p=128),
            )
        nc.sync.dma_start(wo_sb[:, :], wo_d[:, :])
        nc.sync.dma_start(bq_sb[:, :], bq_d[:, :])
        nc.sync.dma_start(bk_sb[:, :], bk_d[:, :])
        nc.sync.dma_start(bvb_sb[:, :], bvb_d[:, :])
        nc.sync.dma_start(id_sb[:, :], id_d[:, :])
        nc.sync.dma_start(mask_sb[:, :], mk_d[:, :])
        nc.gpsimd.memset(ones_bf[:, :], 1.0)
        nc.gpsimd.memset(ones_f32[:, :], 1.0)

        # --- persistent activation tensors ---
        qt_sb = consts.tile([128, BS], dt.bfloat16, name="qt_sb")   # Q^T
        kt_sb = consts.tile([128, BS], dt.bfloat16, name="kt_sb")   # K^T
        vt_sb = consts.tile([128, BS], dt.bfloat16, name="vt_sb")   # V^T
        v_sb = consts.tile([128, BS], dt.bfloat16, name="v_sb")     # V, s-tiles side by side: tile t at cols [t*128, t*128+128)
        ot_sb = consts.tile([128, BS], dt.bfloat16, name="ot_sb")   # attn out ^T

        # single PSUM pool so all phases can interleave:
        #   proj: 2x [128,512] banks (QKV projection accumulators)
        #   st:   2x [128,1024] (scores / bc / out-proj psum)
        #   avden:2x [128,512] (attn-out accumulator + denominators)
        ps = ctx.enter_context(tc.tile_pool(name="ps", bufs=1, space="PSUM"))
        hpool = ctx.enter_context(tc.tile_pool(name="hpool", bufs=10))
        ptpool = ctx.enter_context(tc.tile_pool(name="ptpool", bufs=3))
        rcpool = ctx.enter_context(tc.tile_pool(name="rcpool", bufs=2))
        bcpool = ctx.enter_context(tc.tile_pool(name="bcpool", bufs=2))
        obpool = ctx.enter_context(tc.tile_pool(name="obpool", bufs=3))

        ht_tiles = {}

        def dma_a(bi, split=False):
            hts = []
            for k in range(NK):
                ht_t = hpool.tile([128, S], dt.bfloat16, name=f"ht_{bi}_{k}",
                                  tag="ht", bufs=14)
                if split:
                    # batch 0 gates kernel startup: land the first q-block's
                    # columns early so the first projection chain can start
                    # before the whole batch has arrived
                    for h in range(2):
                        nc.sync.dma_start(
                            ht_t[:, h * (S // 2):(h + 1) * (S // 2)],
                            ht_d[k * 128:(k + 1) * 128,
                                 bi * S + h * (S // 2):bi * S + (h + 1) * (S // 2)])
                else:
                    nc.sync.dma_start(ht_t[:, :], ht_d[k * 128:(k + 1) * 128,
                                                       bi * S:(bi + 1) * S])
                hts.append(ht_t)
            ht_tiles[bi] = hts

        def phase_a_group(bi, sub):
            """QKV projection + V transposes for one 512-token sub-block;
            everything phase_b_j(bi, j=sub) needs beyond earlier subs."""
            hts = ht_tiles[bi]
            cols = slice(bi * S + sub * SQ, bi * S + (sub + 1) * SQ)
            for w_sb, b_sb, o_sb in ((wq_sb, bq_sb, qt_sb),
                                     (wk_sb, bk_sb, kt_sb),
                                     (wv_sb, bvb_sb, vt_sb)):
                pj_ps = ps.tile([128, SQ], dt.float32, tag="proj", bufs=2,
                                name="pj_ps")
                for k in range(NK):
                    nc.tensor.matmul(
                        pj_ps[:, :], w_sb[:, k * 128:(k + 1) * 128],
                        hts[k][:, sub * SQ:(sub + 1) * SQ],
                        start=(k == 0), stop=(k == NK - 1))
                nc.vector.tensor_scalar_add(o_sb[:, cols], pj_ps[:, :],
                                            b_sb[:, 0:1])
            # transpose this sub's V^T back to V [s, hd] via the PE
            for st in range(sub * 4, sub * 4 + 4):
                g = bi * (S // 128) + st
                vtp = ps.tile([128, 128], dt.bfloat16, tag="proj", bufs=2)
                nc.tensor.transpose(
                    vtp[:, :], vt_sb[:, g * 128:(g + 1) * 128], id_sb[:, :])
                nc.vector.tensor_copy(v_sb[:, g * 128:(g + 1) * 128],
                                      vtp[:, :])
            if sub == NJ - 1:
                ht_tiles.pop(bi)

        def phase_b_j(bi, j):
            # attention for the 2 local heads of batch bi, q-block j
            if True:
                qcols = slice(bi * S + j * SQ, bi * S + (j + 1) * SQ)
                nk = 4 * j + 4
                av_ps = ps.tile([128, SQ], dt.float32, tag="avden", bufs=2)
                den_ps = ps.tile([128, SQ], dt.float32, tag="avden", bufs=2)
                # software-pipeline with a lag so the AV matmuls' exp
                # dependencies are already satisfied when the PE reaches
                # them (keeps the PE stream back-to-back)
                LAG = 4
                pts = {}
                for kk in range(nk + LAG):
                    if kk < nk:
                        k = kk
                        kvc = slice(bi * S + k * 128, bi * S + (k + 1) * 128)
                        st_ps = ps.tile([128, 2 * SQ], dt.float32, tag="st",
                                        bufs=2)
                        # scores^T for both heads, row-packed (K=64 each)
                        nc.tensor.matmul(st_ps[:, 0:SQ], kt_sb[0:64, kvc],
                                         qt_sb[0:64, qcols],
                                         start=True, stop=True)
                        nc.tensor.matmul(st_ps[:, SQ:2 * SQ],
                                         kt_sb[64:128, kvc],
                                         qt_sb[64:128, qcols],
                                         start=True, stop=True)
                        pt = ptpool.tile([128, 2 * SQ], dt.bfloat16,
                                         tag="pt", bufs=6)
                        pts[k] = pt
                        delta = (k - (nk - 4)) * 128 if k >= nk - 4 else 0
                        if delta >= 256:
                            # columns < delta are fully masked: skip their
                            # exp (mask multiply below zeroes them; pt slots
                            # only ever hold finite stale values)
                            nc.scalar.activation(pt[:, delta:SQ],
                                                 st_ps[:, delta:SQ], AF.Exp,
                                                 scale=SCALE)
                            nc.scalar.activation(pt[:, SQ + delta:2 * SQ],
                                                 st_ps[:, SQ + delta:2 * SQ],
                                                 AF.Exp, scale=SCALE)
                        else:
                            nc.scalar.activation(pt[:, :], st_ps[:, :],
                                                 AF.Exp, scale=SCALE)
                        if k >= nk - 4:
                            midx = k - (nk - 4)
                            nc.vector.tensor_mul(
                                pt[:, :], pt[:, :],
                                mask_sb[:, midx * 1024:(midx + 1) * 1024])
                        if taps is not None and bi == 0 and j == 0 and k == 0:
                            nc.sync.dma_start(taps["dpt"][:, :], pt[:, :])
                    if kk >= LAG:
                        k = kk - LAG
                        pt = pts.pop(k)
                        # AV col-packed: head A -> rows 0:64, head B -> rows
                        # 64:128 of one bank; denominators -> rows 0 / 32 of
                        # a second bank. Same-bank groups are partition-
                        # disjoint; sim's group check is partition-collapsed
                        # -> skip.
                        g = bi * (S // 128) + k
                        va = v_sb[:, g * 128:g * 128 + 64]
                        vb = v_sb[:, g * 128 + 64:g * 128 + 128]
                        first, last = (k == 0), (k == nk - 1)
                        nc.tensor.matmul(av_ps[0:64, 0:SQ], va, pt[:, 0:SQ],
                                         start=first, stop=last)
                        nc.tensor.matmul(av_ps[64:128, 0:SQ], vb,
                                         pt[:, SQ:2 * SQ],
                                         start=first, stop=last,
                                         skip_group_check=True)
                        nc.tensor.matmul(den_ps[0:1, 0:SQ], ones_bf[:, 0:1],
                                         pt[:, 0:SQ], start=first, stop=last)
                        nc.tensor.matmul(den_ps[32:33, 0:SQ],
                                         ones_bf[:, 0:1], pt[:, SQ:2 * SQ],
                                         start=first, stop=last,
                                         skip_group_check=True)

                if taps is not None and bi == 0 and j == 0:
                    davs = rcpool.tile([128, SQ], dt.float32, tag="dav", bufs=1)
                    nc.scalar.copy(davs[:, :], av_ps[:, :])
                    nc.sync.dma_start(taps["dav"][:, :], davs[:, :])
                    ddens = rcpool.tile([128, SQ], dt.float32, tag="dden", bufs=1)
                    nc.scalar.copy(ddens[:, :], den_ps[:, :])
                    nc.sync.dma_start(taps["dden"][:, :], ddens[:, :])

                # ---- softmax normalization: 1/den = exp(-ln(den)) on ACT
                # (same table set as the softmax exp) ----
                rc = rcpool.tile([128, SQ], dt.float32, tag="rc", bufs=2)
                sc = rcpool.tile([128, SQ], dt.float32, tag="sc", bufs=2)
                nc.scalar.activation(sc[0:1, 0:SQ], den_ps[0:1, 0:SQ], AF.Ln)
                nc.scalar.activation(rc[0:1, 0:SQ], sc[0:1, 0:SQ],
                                     AF.Exp, scale=-1.0)
                nc.scalar.activation(sc[32:33, 0:SQ], den_ps[32:33, 0:SQ],
                                     AF.Ln)
                nc.scalar.activation(rc[32:33, 0:SQ], sc[32:33, 0:SQ],
                                     AF.Exp, scale=-1.0)
                # broadcast 1/den across the head partitions: head A via the
                # (idle) GPSIMD engine -- its partition_broadcast only reads
                # physical partition 0, which is exactly where den_A's recip
                # lives; head B (partition 32) still needs the PE matmul.
                bc_sb = bcpool.tile([128, SQ], dt.float32, tag="bc", bufs=2)
                nc.gpsimd.partition_broadcast(bc_sb[0:64, 0:SQ],
                                              rc[0:1, 0:SQ])
                # head B's broadcast rides in the unused rows 64:128 of the
                # den bank (no extra PSUM slot, no st-slot contention)
                nc.tensor.matmul(den_ps[64:128, 0:SQ], ones_f32[32:33, :],
                                 rc[32:33, 0:SQ], start=True, stop=True,
                                 skip_group_check=True)
                nc.vector.tensor_copy(bc_sb[64:128, 0:SQ],
                                      den_ps[64:128, 0:SQ])
                nc.vector.tensor_mul(ot_sb[0:64, qcols], av_ps[0:64, 0:SQ],
                                     bc_sb[0:64, 0:SQ])
                nc.vector.tensor_mul(ot_sb[64:128, qcols],
                                     av_ps[64:128, 0:SQ],
                                     bc_sb[64:128, 0:SQ])
                if taps is not None and bi == 0 and j == 0:
                    nc.sync.dma_start(taps["drc"][:, :], rc[:, :])
                    nc.sync.dma_start(taps["dbc"][:, :], bc_sb[:, :])

        def phase_c_sp(bi, sp, on_act=False):
            # output projection for one pair of s-tiles of batch bi
            ob = obpool.tile([128, 2048], dt.float32, tag="ob", bufs=3)
            for half in range(2):
                t = bi * (S // 128) + sp * 2 + half
                op_ps = ps.tile([128, 1024], dt.float32, tag="st", bufs=2)
                lhs = ot_sb[:, t * 128:(t + 1) * 128]
                nc.tensor.matmul(op_ps[:, 0:512], lhs, wo_sb[:, 0:512],
                                 start=True, stop=True)
                nc.tensor.matmul(op_ps[:, 512:1024], lhs, wo_sb[:, 512:1024],
                                 start=True, stop=True)
                if on_act and half == 0:
                    # tail units: split the PSUM->SBUF copies across the
                    # by-then idle ScalarE to unblock the DVE
                    nc.scalar.copy(ob[:, 0:1024], op_ps[:, :])
                else:
                    nc.vector.tensor_copy(
                        ob[:, half * 1024:(half + 1) * 1024], op_ps[:, :])
            row0 = bi * S + sp * 256
            nc.sync.dma_start(
                out_d[row0:row0 + 256, :].rearrange("(a p) n -> p a n", p=128),
                ob.rearrange("p (a n) -> p a n", a=2),
            )

        # the first pt slots are read (dead columns) before being fully
        # written; clear them so stale SBUF garbage can't be inf/nan
        for _ in range(6):
            ptz = ptpool.tile([128, 2 * SQ], dt.bfloat16, tag="pt", bufs=6)
            nc.gpsimd.memset(ptz[:, :], 0)



        # Emission plan: the per-engine instruction streams are fixed at
        # compile time, so PE-dense filler must be emitted inside the
        # attention stream where the exp-dependency stalls happen. Each
        # q-block of attention gets exactly one projection sub-group,
        # emitted two q-blocks ahead of its consumer -- uniform filler
        # density across the whole kernel -- plus deferred out-proj units.
        groups = [(bi, sub) for bi in range(B) for sub in range(NJ)]
        dma_a(0, split=True)
        phase_a_group(0, 0)
        phase_a_group(0, 1)
        gidx = 2
        deferred_c = collections.deque()
        for bi in range(B):
            for j in range(NJ):
                if j == 0 and bi + 1 < B:
                    dma_a(bi + 1)
                gj = 4 * bi + j
                while gidx < len(groups) and \
                        groups[gidx][0] * 4 + groups[gidx][1] <= gj + 2:
                    phase_a_group(*groups[gidx])
                    gidx += 1
                phase_b_j(bi, j)
                deferred_c.append((bi, 2 * j))
                deferred_c.append((bi, 2 * j + 1))
                npop = 2 if gj >= 2 else 0
                for _ in range(min(npop, len(deferred_c))):
                    phase_c_sp(*deferred_c.popleft())
        while gidx < len(groups):
            phase_a_group(*groups[gidx])
            gidx += 1
        while deferred_c:
            phase_c_sp(*deferred_c.popleft(), on_act=True)

        if taps is not None:
            nc.sync.dma_start(taps["dqt"][:, :], qt_sb[:, :])
            nc.sync.dma_start(taps["dkt"][:, :], kt_sb[:, :])
            nc.sync.dma_start(taps["dv"][:, :], v_sb[:, :])
            nc.sync.dma_start(taps["dot"][:, :], ot_sb[:, :])


def _get_nc():
    if "nc" not in _CACHE:
        _CACHE["nc"] = _build_nc()
    return _CACHE["nc"]


def _build_mask():
    # mask[kv, q] for the 4 diagonal sub-tiles: delta = 0, 128, 256, 384.
    # allowed iff kv_local <= q_local - delta. Each [128, 512] block is
    # duplicated for the two heads -> [128, 1024] per delta, 4 deltas.
    i = np.arange(128)[:, None]
    q = np.arange(SQ)[None, :]
    blocks = []
    for delta in (0, 128, 256, 384):
        m = (i <= (q - delta)).astype(np.float32)
        blocks.append(np.concatenate([m, m], axis=1))
    return np.concatenate(blocks, axis=1).astype(BF16)


def kernel(hidden_states, Wq, bq, Wk, bk, Wv, bv, Wo, bo):
    global LAST_RESULTS
    from concourse import bass_utils

    nc = _get_nc()

    hid = np.ascontiguousarray(
        np.asarray(hidden_states, dtype=np.float32).reshape(BS, D).T)
    ht = hid.astype(BF16)
    mask = _build_mask()
    Wq = np.asarray(Wq, np.float32)
    Wk = np.asarray(Wk, np.float32)
    Wv = np.asarray(Wv, np.float32)
    Wo = np.asarray(Wo, np.float32)
    bq = np.asarray(bq, np.float32)
    bk = np.asarray(bk, np.float32)
    bv = np.asarray(bv, np.float32)
    bo = np.asarray(bo, np.float32)

    in_maps = []
    for c in range(NCORES):
        sl = slice(DL * c, DL * (c + 1))
        in_maps.append({
            "ht": ht,
            "wq": np.ascontiguousarray(Wq[:, sl]).astype(BF16),
            "wk": np.ascontiguousarray(Wk[:, sl]).astype(BF16),
            "wv": np.ascontiguousarray(Wv[:, sl]).astype(BF16),
            "wo": np.ascontiguousarray(Wo[sl, :]).astype(BF16),
            "bq": np.ascontiguousarray(bq[sl]).reshape(DL, 1),
            "bk": np.ascontiguousarray(bk[sl]).reshape(DL, 1),
            "bvb": np.ascontiguousarray(bv[sl]).reshape(DL, 1),
            "ident": np.eye(128, dtype=np.float32).astype(BF16),
            "mask": mask,
        })

    res = bass_utils.run_bass_kernel_spmd(
        nc, in_maps, core_ids=list(range(NCORES)))
    LAST_RESULTS = res

    out = res.results[0]["out"].astype(np.float32).copy()
    for c in range(1, NCORES):
        out += res.results[c]["out"]
    out += bo[None, :]
    return out.reshape(B, S, D)
